# revision 15
# baseline (speedup 1.0000x reference)
"""Trainium2 Bass kernel for the branched cross-attention processor.

Problem (full shapes):
  hidden_states [4, 4096, 1280], encoder_hidden_states [4, 77, 2048],
  id_embedding [2, 32, 2048], Wq/Wout [1280,1280], Wk/Wv/Wid_k/Wid_v
  [2048,1280], bout [1280].  20 heads, dh=64.  Output [4, 4096, 1280].

Sharding: data-parallel over (batch, seq-half): core c handles batch c//2,
query rows (c%2)*2048 : (c%2+1)*2048.  The K/V projection (109 keys) for a
batch is SPLIT across the core pair that shares it: each core computes 5
of the 10 [*,512] projection chunks (even core: global chunks 0,2,4,6,8;
odd: 1,3,5,7,9) and three pairwise AllGathers exchange the results
progressively, ordered so attention head-pairs unlock in sequence.  This
halves the dominant DMA stream (the 21MB of K/V projection weights) and
removes 80 duplicated matmuls per core.

Schedule (single continuous PE stream to keep the HAM clock gate warm):
  - DMA priority order: wq group 0 + hsT blocks first (the q stream), KV
    weight slabs behind, wout last.  Each weight group is ONE contiguous
    dma_start (the runtime shards every DMA across all 16 engines).
  - Q projection groups j=0..9 with local KV chunks interleaved after
    groups 3..7 and exchanges issued after groups 4, 6, 7 — all K/V data
    has landed before the attention phase begins.
  - Attention processed chunk-major (4 chunks x 512 queries x 10 head
    pairs), software-pipelined, with out-projection units of chunk c-1
    interleaved between pairs so the PE never idles long enough for the
    HAM clock gate to drop to 1.2GHz.
  - Output written fp16 (host upcasts); halves output DMA traffic.

Per (head,chunk) math (identical numerics to the reference):
  scoresT = kT_h^T @ qT_h                [128 keys, 512 q]
  probsT  = exp(0.125*scoresT + gapbias) (gap rows 77:96 -> 0)
  attnT   = v_h^T @ probsT               [64, 512]
  denom   = ones^T @ probsT              (replicated over the head's rows)
  attnT  *= 1/denom
  out     = attnT^T @ Wout + bout
"""

import sys
import types

import numpy as np

# ---------------------------------------------------------------------------
# problem constants (hardcoded; kernel.py must be self-contained)
# ---------------------------------------------------------------------------
B = 4
S = 4096
H = 1280
C = 2048
TE = 77          # encoder tokens
TI = 32          # id tokens
HEADS = 20
DH = 64          # head dim
P = 128
LP = 128         # padded key count: [0:77]=ehs, [77:96]=gap, [96:128]=id
GAP0, GAP1 = TE, P - TI   # 77, 96
SC = 2048        # seq rows per core
NJ = H // P      # 10
NI = C // P      # 16
NCH = SC // 512  # 4 sq-chunks of 512
NT = SC // P     # 16 sq-tiles of 128
SCALE = 1.0 / 8.0
NCORES = 8
MCHUNKS = [(0, 512), (512, 512), (1024, 256)]

# kv chunk (proj, n) computes columns 512n:512n+512 of the [k|v] concat
# ([*, 2560]); proj 0 = encoder tokens (rows 0:77 + zero gap), proj 1 = id
# tokens (rows 96:128).  k = cols 0:1280, v = rest.  Global order chosen so
# attention head-pairs unlock progressively after each exchange.
KV_PLAN = [(0, 0), (1, 0), (0, 2), (1, 2), (0, 3), (1, 3),
           (0, 1), (1, 1), (0, 4), (1, 4)]
# exchanges: (local chunks u_lo..u_hi exchanged, kT transposes unlocked)
# ex0 covers g0-3 -> kTMP cols 0:512 + 1024:1280 final -> kT 0-3, 8, 9
# ex1 covers g4-7 -> kTMP cols 512:1024 final -> kT 4-7
# ex2 covers g8-9 -> v tail
EXCHANGES = [(0, 2, [0, 1, 2, 3, 8, 9]), (2, 4, [4, 5, 6, 7]), (4, 5, [])]

_NC_CACHE = {}


def _kv_dest(nc, proj, n, src2d, kTMP, v_sb):
    """Emit copies/DMAs routing a [128,512] kv-chunk result (full rows,
    src2d(lo, hi, c0, c1) -> AP) into kTMP / v_sb.  Returns list of
    (dst_ap, src_ap)."""
    lo, hi = (0, P) if proj == 0 else (GAP1, P)
    outs = []
    if n < 2:
        outs.append((kTMP[lo:hi, 512 * n:512 * n + 512], src2d(lo, hi, 0, 512)))
    elif n == 2:
        outs.append((kTMP[lo:hi, 1024:1280], src2d(lo, hi, 0, 256)))
        outs.append((v_sb[lo:hi, 0:256], src2d(lo, hi, 256, 512)))
    else:
        v0 = 512 * n - 1280
        outs.append((v_sb[lo:hi, v0:v0 + 512], src2d(lo, hi, 0, 512)))
    return outs


def _ensure_axon_hooks():
    """The image's antenv lacks axon_hooks; synthesize it so NTFF profiling
    (trace=True) works when test.py asks for it.  Harmless if unused."""
    if "antenv.axon_hooks" in sys.modules:
        return
    try:
        import antenv
        from trn_agent_boot.trn_boot import _ntff_profile_via_ctypes

        hook = _ntff_profile_via_ctypes("/opt/axon/libaxon_pjrt.so")
        m = types.ModuleType("antenv.axon_hooks")
        m.get_axon_ntff_profile_hook = lambda: hook
        m.set_axon_ntff_profile_hook = lambda h: None
        sys.modules["antenv.axon_hooks"] = m
        antenv.axon_hooks = m
    except Exception:
        pass


def build_nc():
    """Build + compile the per-core Bass program (SPMD: same NEFF, 8 cores)."""
    if "nc" in _NC_CACHE:
        return _NC_CACHE["nc"]

    import concourse.bass as bass
    import concourse.tile as tile
    from concourse import bacc, mybir
    from concourse.bass import ts

    F32 = mybir.dt.float32
    F16 = mybir.dt.float16
    R = mybir.dt.float16      # matmul operand dtype
    EXP = mybir.ActivationFunctionType.Exp
    PAIR_GROUPS = [[0, 1], [2, 3], [4, 5], [6, 7]]

    nc = bacc.Bacc("TRN2", target_bir_lowering=False, debug=False, num_devices=NCORES)

    hsT = nc.dram_tensor("hsT", [NJ, P, SC], R, kind="ExternalInput").ap()
    xkvTp = nc.dram_tensor("xkvTp", [P, NI * LP], R, kind="ExternalInput").ap()
    wqp = nc.dram_tensor("wqp", [NJ, P, NJ * P], R, kind="ExternalInput").ap()
    wkvp = nc.dram_tensor("wkvp", [5, P, NI * 512], R, kind="ExternalInput").ap()
    woutT = nc.dram_tensor("woutT", [H, H], R, kind="ExternalInput").ap()
    boutb = nc.dram_tensor("boutb", [P, H], F32, kind="ExternalInput").ap()
    out = nc.dram_tensor("out", [SC, H], F16, kind="ExternalOutput").ap()

    with tile.TileContext(nc) as tc:
        with tc.tile_pool(name="pers", bufs=1) as pers:
            # ---- persistent constants / arrays --------------------------------
            ones_mat = pers.tile([P, DH], R, tag="ones_mat")
            nc.vector.memset(ones_mat[:, :], 1.0)
            bias_col = pers.tile([P, 1], F32, tag="bias_col")
            # engine ops need 32-aligned start partitions: write the gap
            # as [64:96] then restore [64:77]; later writes overwrite cleanly.
            nc.vector.memset(bias_col[:, :], 0.0)
            nc.vector.memset(bias_col[64:GAP1, :], -1e30)
            nc.vector.memset(bias_col[64:GAP0, :], 0.0)
            kT_sb = [pers.tile([P, LP], R, tag=f"kT{j}", name=f"kT{j}") for j in range(NJ)]
            v_sb = pers.tile([LP, HEADS * DH], R, tag="v")
            qT_sb = [pers.tile([P, SC], R, tag=f"qT{j}", name=f"qT{j}") for j in range(NJ)]

            # ---- phase Q: q proj + local kv chunks + pairwise exchanges -------
            with (
                tc.tile_pool(name="phq", bufs=1) as phq,
                tc.tile_pool(name="stg", bufs=1) as stg,
                tc.tile_pool(name="wkvs", bufs=2) as wkvs,
                tc.tile_pool(name="wqs", bufs=10) as wqs,
                tc.tile_pool(name="dram", bufs=1, space="DRAM") as dram,
                tc.tile_pool(name="psq", bufs=6, space="PSUM") as psq,
                tc.tile_pool(name="pskv", bufs=2, space="PSUM") as pskv,
            ):
                hsT_sb = [phq.tile([P, SC], R, tag=f"hsT{i}", name=f"hsT{i}")
                          for i in range(NJ)]
                xkvT_sb = phq.tile([P, NI * LP], R, tag="xkvT")
                kTMP = phq.tile([P, H], R, tag="kTMP")
                stage_sb = [stg.tile([P, 512], R, tag=f"st{u}", name=f"st{u}")
                            for u in range(5)]
                inb = dram.tile([5, P, 512], R, tag="inb")
                outb = [dram.tile([2, hi - lo, P, 512], R, tag=f"outb{e}",
                                  name=f"outb{e}")
                        for e, (lo, hi, _) in enumerate(EXCHANGES)]

                wq_t = [None] * NJ

                def fetch_wq(j):
                    wq_t[j] = wqs.tile([P, NJ * P], R, tag="wq", name="wq_t")
                    nc.sync.dma_start(out=wq_t[j][:, :], in_=wqp[j])

                wkv_t = [None] * 5

                def fetch_kv(u):
                    wkv_t[u] = wkvs.tile([P, NI * 512], R, tag="wkv", name="wkv_t")
                    nc.sync.dma_start(out=wkv_t[u][:, :], in_=wkvp[u])

                # DMA priority order: the q stream first, kv weights behind
                wq_sched = {0: [0], 1: [1], 2: [2], 3: [3], 5: [4], 7: [5],
                            9: [6, 7, 8, 9]}
                for i in range(NJ):
                    for j in wq_sched.get(i, []):
                        fetch_wq(j)
                    nc.sync.dma_start(out=hsT_sb[i][:, :], in_=hsT[i])
                nc.sync.dma_start(out=xkvT_sb[:, :], in_=xkvTp)
                fetch_kv(0)
                fetch_kv(1)

                def q_group(j):
                    pss = [psq.tile([P, 512], F32, tag="qps", name="qps")
                           for _ in range(NCH)]
                    for i in range(NJ):
                        for c in range(NCH):
                            nc.tensor.matmul(
                                pss[c][:, :], wq_t[j][:, ts(i, P)],
                                hsT_sb[i][:, ts(c, 512)],
                                start=(i == 0), stop=(i == NJ - 1),
                            )
                    for c in range(NCH):
                        nc.scalar.copy(qT_sb[j][:, ts(c, 512)], pss[c][:, :])

                def kv_chunk_local(u):
                    ps = pskv.tile([P, 512], F32, tag="kvps", name="kvps")
                    for i in range(NI):
                        nc.tensor.matmul(
                            ps[:, :], xkvT_sb[:, ts(i, LP)], wkv_t[u][:, ts(i, 512)],
                            start=(i == 0), stop=(i == NI - 1),
                        )
                    nc.scalar.copy(stage_sb[u][:, :], ps[:, :])
                    nc.sync.dma_start(out=inb[u], in_=stage_sb[u][:, :])

                def exchange(e):
                    u_lo, u_hi, kts = EXCHANGES[e]
                    nc.gpsimd.collective_compute(
                        "AllGather", mybir.AluOpType.bypass,
                        replica_groups=PAIR_GROUPS,
                        ins=[inb[u_lo:u_hi].opt()],
                        outs=[outb[e][:, :, :, :].opt()],
                    )
                    for g in range(2 * u_lo, 2 * u_hi):
                        rank, u = g % 2, g // 2
                        proj, n = KV_PLAN[g]
                        src = outb[e][rank, u - u_lo]

                        def src2d(lo, hi, c0, c1):
                            return src[lo:hi, c0:c1]

                        for dst_ap, src_ap in _kv_dest(nc, proj, n, src2d, kTMP, v_sb):
                            nc.sync.dma_start(out=dst_ap, in_=src_ap)
                    for j in kts:
                        nc.sync.dma_start(out=kT_sb[j][:, :],
                                          in_=kTMP[:, ts(j, P)], transpose=True)

                # local kv chunks after q groups 4..8; exchanges after 5, 7, 8
                kv_at = {4: [0], 5: [1], 6: [2], 7: [3], 8: [4]}
                ex_at = {5: [0], 7: [1], 8: [2]}
                for j in range(NJ):
                    q_group(j)
                    for u in kv_at.get(j, []):
                        kv_chunk_local(u)
                        if u + 2 < 5:
                            fetch_kv(u + 2)
                    for e in ex_at.get(j, []):
                        exchange(e)

            # ---- attention + out-projection, interleaved ----------------------
            attnp_cm = tc.tile_pool(name="attnp", bufs=1)
            attnp = attnp_cm.__enter__()
            attnT_sb = [attnp.tile([P, SC], R, tag=f"attnT{d}", name=f"attnT{d}")
                        for d in range(NJ)]
            # wout / bias stream in behind the kv weights and exchanges
            boutb_sb = attnp.tile([P, H], F32, tag="boutb")
            wout_sb = [attnp.tile([P, H], R, tag=f"wout{i}", name=f"wout{i}")
                       for i in range(NJ)]
            nc.sync.dma_start(out=boutb_sb[:, :], in_=boutb)
            for i in range(NJ):
                nc.sync.dma_start(out=wout_sb[i][:, :], in_=woutT[ts(i, P), :])

            pha_cm = tc.tile_pool(name="pha", bufs=3)
            pha = pha_cm.__enter__()
            psa_cm = tc.tile_pool(name="psa", bufs=1, space="PSUM")
            psa = psa_cm.__enter__()
            pso_cm = tc.tile_pool(name="pso", bufs=1, space="PSUM")
            pso = pso_cm.__enter__()
            fino_cm = tc.tile_pool(name="fino", bufs=3)
            fino = fino_cm.__enter__()

            astate = {}

            def attn_front(c, hp):
                pts = []
                for s in range(2):
                    rq = DH * s
                    ps_s = psa.tile([P, 512], F32, tag=f"sps{s}", name="sps")
                    nc.tensor.matmul(
                        ps_s[:, :], kT_sb[hp][rq:rq + DH, :],
                        qT_sb[hp][rq:rq + DH, ts(c, 512)],
                        start=True, stop=True,
                    )
                    pts.append(ps_s)
                probs = []
                for s in range(2):
                    probsT = pha.tile([P, 512], R, tag="probsT", name="probsT")
                    nc.scalar.activation(
                        probsT[:, :], pts[s][:, :], EXP,
                        bias=bias_col[:, :], scale=SCALE,
                    )
                    probs.append(probsT)
                astate[(c, hp)] = probs

            def attn_back(c, hp):
                probs = astate.pop((c, hp))
                ps_o = psa.tile([P, 512], F32, tag="ops", name="ops")
                ps_d = psa.tile([P, 512], F32, tag="dps", name="dps")
                for s in range(2):
                    h = 2 * hp + s
                    rq = DH * s
                    nc.tensor.matmul(
                        ps_o[rq:rq + DH, :], v_sb[:, ts(h, DH)], probs[s][:, :],
                        start=True, stop=True,
                    )
                    nc.tensor.matmul(
                        ps_d[rq:rq + DH, :], ones_mat[:, :], probs[s][:, :],
                        start=True, stop=True,
                    )
                bc_sb = pha.tile([P, 512], F32, tag="bc", name="bc_sb")
                nc.vector.reciprocal_approx_fast(bc_sb[:, :], ps_d[:, :])
                nc.vector.tensor_mul(
                    attnT_sb[hp][:, ts(c, 512)], ps_o[:, :], bc_sb[:, :]
                )

            fin_t = {}
            psf_t = {}

            def out_unit(t, i):
                # out-projection for seq tile t, contraction block i
                if i == 0:
                    fin_t[t] = fino.tile([P, H], F16, tag="fin", name="fin")
                    psf_t[t] = [
                        pso.tile([P, mw], F32, tag=f"psf{m}", name="psf")
                        for m, (m0, mw) in enumerate(MCHUNKS)
                    ]
                for m, (m0, mw) in enumerate(MCHUNKS):
                    nc.tensor.matmul(
                        psf_t[t][m][:, :], attnT_sb[i][:, ts(t, P)],
                        wout_sb[i][:, m0:m0 + mw],
                        start=(i == 0), stop=(i == NJ - 1),
                    )
                if i == NJ - 1:
                    fin = fin_t.pop(t)
                    psf = psf_t.pop(t)
                    for m, (m0, mw) in enumerate(MCHUNKS):
                        nc.vector.tensor_add(
                            fin[:, m0:m0 + mw], psf[m][:, :],
                            boutb_sb[:, m0:m0 + mw]
                        )
                    nc.sync.dma_start(out=out[ts(t, P), :], in_=fin[:, :])

            # software pipeline over pairs, with out-proj units of the
            # previous chunk (4 per pair) interleaved to keep PE dense.
            pairs = [(c, hp) for c in range(NCH) for hp in range(NJ)]
            units = []  # (t, i) out-proj work queue, filled per chunk

            def interleave_units(k):
                for _ in range(k):
                    if units:
                        out_unit(*units.pop(0))

            for idx, (c, hp) in enumerate(pairs):
                if hp == 0 and c >= 1:
                    # queue out-proj for the 4 seq tiles of chunk c-1
                    units.extend([(t, i) for t in range(4 * (c - 1), 4 * c)
                                  for i in range(NJ)])
                attn_front(c, hp)
                if idx >= 1:
                    attn_back(*pairs[idx - 1])
                interleave_units(4)
            attn_back(*pairs[-1])
            units.extend([(t, i) for t in range(12, 16) for i in range(NJ)])
            interleave_units(len(units))

            fino_cm.__exit__(None, None, None)
            pso_cm.__exit__(None, None, None)
            psa_cm.__exit__(None, None, None)
            pha_cm.__exit__(None, None, None)
            attnp_cm.__exit__(None, None, None)

    nc.compile()
    _NC_CACHE["nc"] = nc
    return nc


def prep_core_inputs(hidden_states, encoder_hidden_states, id_embedding,
                     Wq, Wk, Wv, Wid_k, Wid_v, Wout, bout):
    """Host-side sharding / layout prep.  Returns list of 8 in_maps."""
    f = np.float32
    h16 = np.float16
    hidden_states = np.asarray(hidden_states, f)
    encoder_hidden_states = np.asarray(encoder_hidden_states, f)
    id_embedding = np.asarray(id_embedding, f)
    Wq = np.asarray(Wq, f)
    Wout = np.asarray(Wout, f)
    Wk, Wv = np.asarray(Wk, f), np.asarray(Wv, f)
    Wid_k, Wid_v = np.asarray(Wid_k, f), np.asarray(Wid_v, f)
    boutb = np.ascontiguousarray(np.broadcast_to(np.asarray(bout, f), (P, H)))

    # packed weight layouts: one contiguous DMA per group
    # wqp[j, p, i*128+c] = Wq[i*128+p, j*128+c]
    wqp = np.ascontiguousarray(
        Wq.reshape(NJ, P, NJ, P).transpose(2, 1, 0, 3)
        .reshape(NJ, P, NJ * P).astype(h16))
    # kv weight slabs: global chunk g=(proj,n) -> [p, i*512+c] = W[i*128+p, 512n+c];
    # core with parity r gets global chunks r, r+2, ... as its 5 local slabs
    wkv = np.concatenate([Wk, Wv], axis=1).reshape(NI, P, 5, 512)
    widkv = np.concatenate([Wid_k, Wid_v], axis=1).reshape(NI, P, 5, 512)
    wkvp_par = []
    for r in range(2):
        slabs = np.empty((5, P, NI * 512), h16)
        for u in range(5):
            proj, n = KV_PLAN[2 * u + r]
            src = wkv if proj == 0 else widkv
            slabs[u] = src[:, :, n, :].transpose(1, 0, 2).reshape(P, NI * 512)
        wkvp_par.append(slabs)

    wout16 = np.ascontiguousarray(Wout.astype(h16))
    in_maps = []
    for core in range(NCORES):
        b, hf = divmod(core, 2)
        hsT = np.ascontiguousarray(
            hidden_states[b, hf * SC:(hf + 1) * SC, :].T.astype(h16)
        ).reshape(NJ, P, SC)
        xkvT = np.zeros((C, LP), h16)                                          # [C, 128]
        xkvT[:, :TE] = encoder_hidden_states[b].T
        xkvT[:, GAP1:] = id_embedding[b % 2].T
        # [i, p, l] -> [p, i*128+l]
        xkvTp = np.ascontiguousarray(
            xkvT.reshape(NI, P, LP).transpose(1, 0, 2).reshape(P, NI * LP))
        in_maps.append({
            "hsT": hsT, "xkvTp": xkvTp, "wqp": wqp, "wkvp": wkvp_par[core % 2],
            "woutT": wout16, "boutb": boutb,
        })
    return in_maps


def kernel(hidden_states, encoder_hidden_states, id_embedding,
           Wq, Wk, Wv, Wid_k, Wid_v, Wout, bout, _trace=False):
    _ensure_axon_hooks()
    from concourse.bass_utils import run_bass_kernel_spmd

    nc = build_nc()
    in_maps = prep_core_inputs(hidden_states, encoder_hidden_states, id_embedding,
                               Wq, Wk, Wv, Wid_k, Wid_v, Wout, bout)
    kwargs = {}
    if _trace:
        import concourse.bass_utils as bu
        bu.upload_artifacts = lambda tmpdir: f"local://{tmpdir}"
        kwargs["trace"] = True
    res = run_bass_kernel_spmd(nc, in_maps, core_ids=list(range(NCORES)), **kwargs)

    outp = np.empty((B, S, H), np.float32)
    for core in range(NCORES):
        b, hf = divmod(core, 2)
        outp[b, hf * SC:(hf + 1) * SC, :] = res.results[core]["out"].astype(np.float32)
    if _trace:
        kernel.last_exec_time_ns = res.exec_time_ns
        kernel.last_results = res
    return outp


# revision 19
# speedup vs baseline: 1.0837x; 1.0837x over previous
"""Trainium2 Bass kernel for the branched cross-attention processor.

Problem (full shapes):
  hidden_states [4, 4096, 1280], encoder_hidden_states [4, 77, 2048],
  id_embedding [2, 32, 2048], Wq/Wout [1280,1280], Wk/Wv/Wid_k/Wid_v
  [2048,1280], bout [1280].  20 heads, dh=64.  Output [4, 4096, 1280].

Sharding: data-parallel over (batch, seq-half): core c handles batch c//2,
query rows (c%2)*2048 : (c%2+1)*2048.  K/V (109 keys) are computed
per-core for its batch.  All queries are independent (full cross
attention), so no collectives are needed.

Schedule (single continuous PE stream to keep the HAM clock gate warm):
  - DMA priority order: Wq slabs interleaved with hsT blocks first (the q
    stream), KV-projection weight half-slabs behind them, Wout last.
    Every weight group is one contiguous dma_start (the runtime shards
    each DMA across all 16 engines, so big transfers lose no parallelism
    and cut sync-engine issue time 16x).
  - Q projection groups j=0..9 with one KV chunk interleaved after each
    of groups 2..9; the last two KV chunks (v columns 768:1280) run
    inside attention chunk 0, overlapping their weight DMA tail.
  - kT transposes issued as soon as their k columns are final.
  - Attention processed chunk-major (4 chunks x 512 queries x 10 head
    pairs), software-pipelined (scores+exp of pair p overlap PV/denom/
    normalize of pair p-1), with out-projection units of chunk c-1
    interleaved between pairs so the PE never idles long enough for the
    HAM clock gate to drop to 1.2GHz.
  - Output written fp16 (host upcasts); halves output DMA traffic.

Per (head,chunk) math (identical numerics to the reference):
  scoresT = kT_h^T @ qT_h                [128 keys, 512 q]
  probsT  = exp(0.125*scoresT + gapbias) (gap rows 77:96 -> 0)
  attnT   = v_h^T @ probsT               [64, 512]
  denom   = ones^T @ probsT              (replicated over the head's rows)
  attnT  *= 1/denom
  out     = attnT^T @ Wout + bout
"""

import sys
import types

import numpy as np

# ---------------------------------------------------------------------------
# problem constants (hardcoded; kernel.py must be self-contained)
# ---------------------------------------------------------------------------
B = 4
S = 4096
H = 1280
C = 2048
TE = 77          # encoder tokens
TI = 32          # id tokens
HEADS = 20
DH = 64          # head dim
P = 128
LP = 128         # padded key count: [0:77]=ehs, [77:96]=gap, [96:128]=id
GAP0, GAP1 = TE, P - TI   # 77, 96
SC = 2048        # seq rows per core
NJ = H // P      # 10
NI = C // P      # 16
NCH = SC // 512  # 4 sq-chunks of 512
NT = SC // P     # 16 sq-tiles of 128
SCALE = 1.0 / 8.0
NCORES = 8
MCHUNKS = [(0, 512), (512, 512), (1024, 256)]

# kv chunk t=(proj, n) computes columns 512n:512n+512 of the [k|v] concat
# ([*, 2560]); proj 0 = encoder tokens (rows 0:77 + zero gap), proj 1 = id
# tokens (rows 96:128).  k = cols 0:1280, v = rest.  k chunks first so the
# kT transposes can start early; chunks 8, 9 run inside attention chunk 0.
KV_PLAN = [(0, 0), (0, 1), (1, 0), (0, 2), (1, 1), (1, 2),
           (0, 3), (1, 3), (0, 4), (1, 4)]
# kT transpose groups: after KV_PLAN index -> list of kT blocks final
KT_AT = {2: [0, 1, 2, 3], 4: [4, 5, 6, 7], 5: [8, 9]}
# kv chunks interleaved after q_group j (indices into KV_PLAN)
KV_AT_Q = {2: [0], 3: [1], 4: [2], 5: [3], 6: [4], 7: [5], 8: [6], 9: [7]}
# kv chunks interleaved inside attention chunk 0, after pair hp
KV_AT_A0 = {0: [8], 2: [9]}

_NC_CACHE = {}


def _ensure_axon_hooks():
    """The image's antenv lacks axon_hooks; synthesize it so NTFF profiling
    (trace=True) works when test.py asks for it.  Harmless if unused."""
    if "antenv.axon_hooks" in sys.modules:
        return
    try:
        import antenv
        from trn_agent_boot.trn_boot import _ntff_profile_via_ctypes

        hook = _ntff_profile_via_ctypes("/opt/axon/libaxon_pjrt.so")
        m = types.ModuleType("antenv.axon_hooks")
        m.get_axon_ntff_profile_hook = lambda: hook
        m.set_axon_ntff_profile_hook = lambda h: None
        sys.modules["antenv.axon_hooks"] = m
        antenv.axon_hooks = m
    except Exception:
        pass


def build_nc():
    """Build + compile the per-core Bass program (SPMD: same NEFF, 8 cores)."""
    if "nc" in _NC_CACHE:
        return _NC_CACHE["nc"]

    import concourse.bass as bass
    import concourse.tile as tile
    from concourse import bacc, mybir
    from concourse.bass import ts

    F32 = mybir.dt.float32
    F16 = mybir.dt.float16
    R = mybir.dt.float16      # matmul operand dtype
    EXP = mybir.ActivationFunctionType.Exp

    nc = bacc.Bacc("TRN2", target_bir_lowering=False, debug=False, num_devices=NCORES)

    hsT = nc.dram_tensor("hsT", [NJ, P, SC], R, kind="ExternalInput").ap()
    xkvTp = nc.dram_tensor("xkvTp", [P, NI * LP], R, kind="ExternalInput").ap()
    wqp = nc.dram_tensor("wqp", [NJ, P, NJ * P], R, kind="ExternalInput").ap()
    # kv weights as 20 half-slabs: [2t+h] = chunk t, contraction half h
    wkvp = nc.dram_tensor("wkvp", [20, P, 8 * 512], R, kind="ExternalInput").ap()
    woutT = nc.dram_tensor("woutT", [H, H], R, kind="ExternalInput").ap()
    boutb = nc.dram_tensor("boutb", [P, H], F32, kind="ExternalInput").ap()
    out = nc.dram_tensor("out", [SC, H], F16, kind="ExternalOutput").ap()

    with tile.TileContext(nc) as tc:
        with (
            tc.tile_pool(name="pers", bufs=1) as pers,
            tc.tile_pool(name="wkvs", bufs=5) as wkvs,
        ):
            # ---- persistent constants / arrays --------------------------------
            ones_mat = pers.tile([P, DH], R, tag="ones_mat")
            nc.vector.memset(ones_mat[:, :], 1.0)
            bias_col = pers.tile([P, 1], F32, tag="bias_col")
            # engine ops need 32-aligned start partitions: write the gap
            # as [64:96] then restore [64:77]; later writes overwrite cleanly.
            nc.vector.memset(bias_col[:, :], 0.0)
            nc.vector.memset(bias_col[64:GAP1, :], -1e30)
            nc.vector.memset(bias_col[64:GAP0, :], 0.0)
            kT_sb = [pers.tile([P, LP], R, tag=f"kT{j}", name=f"kT{j}") for j in range(NJ)]
            v_sb = pers.tile([LP, HEADS * DH], R, tag="v")
            qT_sb = [pers.tile([P, SC], R, tag=f"qT{j}", name=f"qT{j}") for j in range(NJ)]
            xkvT_sb = pers.tile([P, NI * LP], R, tag="xkvT")

            wkv_t = [None] * 20

            def fetch_kv_half(th):
                if th >= 20 or wkv_t[th] is not None:
                    return
                wkv_t[th] = wkvs.tile([P, 8 * 512], R, tag="wkv", name="wkv_t")
                nc.sync.dma_start(out=wkv_t[th][:, :], in_=wkvp[th])

            kv_ps_pool = [None]  # set per phase (pskv, then psa)
            kv_ps_tag = [None]

            def kv_chunk(t):
                proj, n = KV_PLAN[t]
                ps = kv_ps_pool[0].tile([P, 512], F32, tag=kv_ps_tag[0], name="kvps")
                for h in range(2):
                    for i in range(8):
                        nc.tensor.matmul(
                            ps[:, :], xkvT_sb[:, ts(8 * h + i, LP)],
                            wkv_t[2 * t + h][:, ts(i, 512)],
                            start=(h == 0 and i == 0), stop=(h == 1 and i == 7),
                        )
                lo, hi = (0, P) if proj == 0 else (GAP1, P)
                if n < 2:
                    nc.scalar.copy(kTMP[lo:hi, ts(n, 512)], ps[lo:hi, :])
                elif n == 2:
                    nc.scalar.copy(kTMP[lo:hi, 1024:1280], ps[lo:hi, 0:256])
                    nc.scalar.copy(v_sb[lo:hi, 0:256], ps[lo:hi, 256:512])
                else:
                    v0 = 512 * n - 1280
                    nc.scalar.copy(v_sb[lo:hi, v0:v0 + 512], ps[lo:hi, :])
                for j in KT_AT.get(t, []):
                    nc.sync.dma_start(out=kT_sb[j][:, :],
                                      in_=kTMP[:, ts(j, P)], transpose=True)

            # ---- phase Q: q projection + k/v projection, interleaved ----------
            with (
                tc.tile_pool(name="phq", bufs=1) as phq,
                tc.tile_pool(name="wqs", bufs=10) as wqs,
                tc.tile_pool(name="psq", bufs=6, space="PSUM") as psq,
                tc.tile_pool(name="pskv", bufs=2, space="PSUM") as pskv,
            ):
                hsT_sb = [phq.tile([P, SC], R, tag=f"hsT{i}", name=f"hsT{i}")
                          for i in range(NJ)]
                kTMP = phq.tile([P, H], R, tag="kTMP")

                wq_t = [None] * NJ

                def fetch_wq(j):
                    wq_t[j] = wqs.tile([P, NJ * P], R, tag="wq", name="wq_t")
                    nc.sync.dma_start(out=wq_t[j][:, :], in_=wqp[j])

                # DMA priority order: the q stream first, kv weights behind.
                # hsT block 0 is split so the very first matmul can start on
                # a 128KB transfer instead of 512KB.
                fetch_wq(0)
                nc.sync.dma_start(out=hsT_sb[0][:, 0:512], in_=hsT[0][:, 0:512])
                nc.sync.dma_start(out=hsT_sb[0][:, 512:SC], in_=hsT[0][:, 512:SC])
                wq_sched = {1: [1], 2: [2], 3: [3], 5: [4], 7: [5],
                            9: [6, 7, 8, 9]}
                for i in range(1, NJ):
                    for j in wq_sched.get(i, []):
                        fetch_wq(j)
                    nc.sync.dma_start(out=hsT_sb[i][:, :], in_=hsT[i])
                nc.sync.dma_start(out=xkvT_sb[:, :], in_=xkvTp)
                fetch_kv_half(0)
                fetch_kv_half(1)
                fetch_kv_half(2)

                def q_group(j):
                    pss = [psq.tile([P, 512], F32, tag="qps", name="qps")
                           for _ in range(NCH)]
                    for i in range(NJ):
                        for c in range(NCH):
                            nc.tensor.matmul(
                                pss[c][:, :], wq_t[j][:, ts(i, P)],
                                hsT_sb[i][:, ts(c, 512)],
                                start=(i == 0), stop=(i == NJ - 1),
                            )
                    for c in range(NCH):
                        nc.scalar.copy(qT_sb[j][:, ts(c, 512)], pss[c][:, :])

                kv_ps_pool[0], kv_ps_tag[0] = pskv, "kvps"
                for j in range(NJ):
                    q_group(j)
                    for t in KV_AT_Q.get(j, []):
                        fetch_kv_half(2 * t + 3)
                        fetch_kv_half(2 * t + 4)
                        kv_chunk(t)

            # ---- attention + out-projection, interleaved (right-side pools) ---
            attnp_cm = tc.tile_pool(name="attnp", bufs=1, side="right")
            attnp = attnp_cm.__enter__()
            attnT_sb = [attnp.tile([P, SC], R, tag=f"attnT{d}", name=f"attnT{d}")
                        for d in range(NJ)]
            boutb_sb = attnp.tile([P, H], F32, tag="boutb")
            wout_sb = [attnp.tile([P, H], R, tag=f"wout{i}", name=f"wout{i}")
                       for i in range(NJ)]
            nc.sync.dma_start(out=boutb_sb[:, :], in_=boutb)
            for i in range(NJ):
                nc.sync.dma_start(out=wout_sb[i][:, :], in_=woutT[ts(i, P), :])

            pha_cm = tc.tile_pool(name="pha", bufs=3, side="right")
            pha = pha_cm.__enter__()
            fino_cm = tc.tile_pool(name="fino", bufs=3, side="right")
            fino = fino_cm.__enter__()
            psa_cm = tc.tile_pool(name="psa", bufs=1, space="PSUM")
            psa = psa_cm.__enter__()
            pso_cm = tc.tile_pool(name="pso", bufs=1, space="PSUM")
            pso = pso_cm.__enter__()

            astate = {}

            def attn_front(c, hp):
                pts = []
                for s in range(2):
                    rq = DH * s
                    ps_s = psa.tile([P, 512], F32, tag=f"sps{s}", name="sps")
                    nc.tensor.matmul(
                        ps_s[:, :], kT_sb[hp][rq:rq + DH, :],
                        qT_sb[hp][rq:rq + DH, ts(c, 512)],
                        start=True, stop=True,
                    )
                    pts.append(ps_s)
                probs = []
                for s in range(2):
                    probsT = pha.tile([P, 512], R, tag="probsT", name="probsT")
                    nc.scalar.activation(
                        probsT[:, :], pts[s][:, :], EXP,
                        bias=bias_col[:, :], scale=SCALE,
                    )
                    probs.append(probsT)
                astate[(c, hp)] = probs

            def attn_back(c, hp):
                probs = astate.pop((c, hp))
                ps_o = psa.tile([P, 512], F32, tag="ops", name="ops")
                ps_d = psa.tile([P, 512], F32, tag="dps", name="dps")
                for s in range(2):
                    h = 2 * hp + s
                    rq = DH * s
                    nc.tensor.matmul(
                        ps_o[rq:rq + DH, :], v_sb[:, ts(h, DH)], probs[s][:, :],
                        start=True, stop=True,
                    )
                    nc.tensor.matmul(
                        ps_d[rq:rq + DH, :], ones_mat[:, :], probs[s][:, :],
                        start=True, stop=True,
                    )
                bc_sb = pha.tile([P, 512], F32, tag="bc", name="bc_sb")
                nc.vector.reciprocal_approx_fast(bc_sb[:, :], ps_d[:, :])
                nc.vector.tensor_mul(
                    attnT_sb[hp][:, ts(c, 512)], ps_o[:, :], bc_sb[:, :]
                )

            fin_t = {}
            psf_t = {}

            def out_unit(t, i):
                # out-projection for seq tile t, contraction block i
                if i == 0:
                    fin_t[t] = fino.tile([P, H], F16, tag="fin", name="fin")
                    psf_t[t] = [
                        pso.tile([P, mw], F32, tag=f"psf{m}", name="psf")
                        for m, (m0, mw) in enumerate(MCHUNKS)
                    ]
                for m, (m0, mw) in enumerate(MCHUNKS):
                    nc.tensor.matmul(
                        psf_t[t][m][:, :], attnT_sb[i][:, ts(t, P)],
                        wout_sb[i][:, m0:m0 + mw],
                        start=(i == 0), stop=(i == NJ - 1),
                    )
                if i == NJ - 1:
                    fin = fin_t.pop(t)
                    psf = psf_t.pop(t)
                    for m, (m0, mw) in enumerate(MCHUNKS):
                        nc.vector.tensor_add(
                            fin[:, m0:m0 + mw], psf[m][:, :],
                            boutb_sb[:, m0:m0 + mw]
                        )
                    nc.sync.dma_start(out=out[ts(t, P), :], in_=fin[:, :])

            # software pipeline over pairs, with out-proj units of the
            # previous chunk (4 per pair) interleaved to keep PE dense.
            pairs = [(c, hp) for c in range(NCH) for hp in range(NJ)]
            units = []  # (t, i) out-proj work queue, filled per chunk

            def interleave_units(k):
                for _ in range(k):
                    if units:
                        out_unit(*units.pop(0))

            kv_ps_pool[0], kv_ps_tag[0] = psa, "kvo"
            for idx, (c, hp) in enumerate(pairs):
                if hp == 0 and c >= 1:
                    # queue out-proj for the 4 seq tiles of chunk c-1
                    units.extend([(t, i) for t in range(4 * (c - 1), 4 * c)
                                  for i in range(NJ)])
                attn_front(c, hp)
                if idx >= 1:
                    attn_back(*pairs[idx - 1])
                if c == 0:
                    for t in KV_AT_A0.get(hp, []):
                        fetch_kv_half(2 * t + 2)
                        fetch_kv_half(2 * t + 3)
                        kv_chunk(t)
                interleave_units(4)
            attn_back(*pairs[-1])
            units.extend([(t, i) for t in range(12, 16) for i in range(NJ)])
            interleave_units(len(units))

            pso_cm.__exit__(None, None, None)
            psa_cm.__exit__(None, None, None)
            fino_cm.__exit__(None, None, None)
            pha_cm.__exit__(None, None, None)
            attnp_cm.__exit__(None, None, None)

    nc.compile()
    _NC_CACHE["nc"] = nc
    return nc


def prep_core_inputs(hidden_states, encoder_hidden_states, id_embedding,
                     Wq, Wk, Wv, Wid_k, Wid_v, Wout, bout):
    """Host-side sharding / layout prep.  Returns list of 8 in_maps."""
    f = np.float32
    h16 = np.float16
    hidden_states = np.asarray(hidden_states, f)
    encoder_hidden_states = np.asarray(encoder_hidden_states, f)
    id_embedding = np.asarray(id_embedding, f)
    Wq = np.asarray(Wq, f)
    Wout = np.asarray(Wout, f)
    Wk, Wv = np.asarray(Wk, f), np.asarray(Wv, f)
    Wid_k, Wid_v = np.asarray(Wid_k, f), np.asarray(Wid_v, f)
    boutb = np.ascontiguousarray(np.broadcast_to(np.asarray(bout, f), (P, H)))

    # packed weight layouts: one contiguous DMA per group
    # wqp[j, p, i*128+c] = Wq[i*128+p, j*128+c]
    wqp = np.ascontiguousarray(
        Wq.reshape(NJ, P, NJ, P).transpose(2, 1, 0, 3)
        .reshape(NJ, P, NJ * P).astype(h16))
    # kv weight half-slabs: [2t+h][p, i*512+c] = W[(8h+i)*128+p, 512n+c]
    wkv = np.concatenate([Wk, Wv], axis=1).reshape(NI, P, 5, 512)
    widkv = np.concatenate([Wid_k, Wid_v], axis=1).reshape(NI, P, 5, 512)
    wkvp = np.empty((20, P, 8 * 512), h16)
    for t, (proj, n) in enumerate(KV_PLAN):
        src = wkv if proj == 0 else widkv
        for h in range(2):
            wkvp[2 * t + h] = (src[8 * h:8 * h + 8, :, n, :]
                               .transpose(1, 0, 2).reshape(P, 8 * 512))

    wout16 = np.ascontiguousarray(Wout.astype(h16))
    in_maps = []
    for core in range(NCORES):
        b, hf = divmod(core, 2)
        hsT = np.ascontiguousarray(
            hidden_states[b, hf * SC:(hf + 1) * SC, :].T.astype(h16)
        ).reshape(NJ, P, SC)
        xkvT = np.zeros((C, LP), h16)                                          # [C, 128]
        xkvT[:, :TE] = encoder_hidden_states[b].T
        xkvT[:, GAP1:] = id_embedding[b % 2].T
        # [i, p, l] -> [p, i*128+l]
        xkvTp = np.ascontiguousarray(
            xkvT.reshape(NI, P, LP).transpose(1, 0, 2).reshape(P, NI * LP))
        in_maps.append({
            "hsT": hsT, "xkvTp": xkvTp, "wqp": wqp, "wkvp": wkvp,
            "woutT": wout16, "boutb": boutb,
        })
    return in_maps


def kernel(hidden_states, encoder_hidden_states, id_embedding,
           Wq, Wk, Wv, Wid_k, Wid_v, Wout, bout, _trace=False):
    _ensure_axon_hooks()
    from concourse.bass_utils import run_bass_kernel_spmd

    nc = build_nc()
    in_maps = prep_core_inputs(hidden_states, encoder_hidden_states, id_embedding,
                               Wq, Wk, Wv, Wid_k, Wid_v, Wout, bout)
    kwargs = {}
    if _trace:
        import concourse.bass_utils as bu
        bu.upload_artifacts = lambda tmpdir: f"local://{tmpdir}"
        kwargs["trace"] = True
    res = run_bass_kernel_spmd(nc, in_maps, core_ids=list(range(NCORES)), **kwargs)

    outp = np.empty((B, S, H), np.float32)
    for core in range(NCORES):
        b, hf = divmod(core, 2)
        outp[b, hf * SC:(hf + 1) * SC, :] = res.results[core]["out"].astype(np.float32)
    if _trace:
        kernel.last_exec_time_ns = res.exec_time_ns
        kernel.last_results = res
    return outp
